# revision 10
# baseline (speedup 1.0000x reference)
"""MultiHeadSelfAttention2D Trainium2 kernel (8-core data parallel over batch).

Math (per batch element b, head n, with L=8, S=L*L=64, DPH=32):
  q = x·Wq*s, k = x·Wk, v = x·Wv  (channel matmuls, spatial flattened)
  logits[ij,pq] = q[ij]·k[pq] + q[ij]·z[ij,pq]
  z[ij,pq,:] = pad_post(emb_h[p-i+7]) + pad_pre(emb_w[q-j+7])
  out = Wo^T · concat_heads(softmax(logits) @ v)

The relative-position term decomposes: q·z = A2[ij, p-i+7] + B2[ij, q-j+7]
with A2[ij,r] = sum_{d<16} q[ij,d] emb_h[r,d], B2 analogous on emb_w.
On device we compute "skewed" projections directly from x:
  skewH[u, (b,i,j)] = A2[b,ij, u+7-i] = x·WH_i[:, (n,u)]   (u in 0..7, per i)
  skewW[v, (b,i,j)] = B2[b,ij, v+7-j] = x·WW_j[:, (n,v)]   (v in 0..7, per j)
so logits^T[pq, ij] = k^T·q (CC) + Sel_n^T·skew (CP), Sel_n constant with
SelH[u,pq]=δ(u==p), SelW[v,pq]=δ(v==q) placed in head n's skew rows.

Layouts (per core, B'=128, blocks of 32 b):
  All "free" dims are (b, s) b-major.  qT/kT/vT: [d(4 heads x 32)·2 chunks,
  (b,s)].  skew: [H: n*8+u (64 rows) | W: 64+n*8+v, (b,ij)].  Logits psum
  windows [2*64 pq (parity b%2), 8 slots=heads * 64 ij] per b-pair.
  E=exp(logits) -> ones-matmul column sums -> reciprocal -> E_norm.
  vP = PE-transposed v: [(b%2)*64+pq, (bpair, chunk, 4 heads*32 d)].
  attnT psum [4n*32 d, 8b*64 ij] -> O-projection -> DRAM.
"""

import sys
import numpy as np

sys.path.insert(0, '/opt/trn_rl_repo')

import ml_dtypes  # noqa: E402
import concourse.bass as bass  # noqa: E402
import concourse.tile as tile  # noqa: E402
import concourse.mybir as mybir  # noqa: E402
from concourse import bacc  # noqa: E402
from concourse.bass_utils import run_bass_kernel_spmd  # noqa: E402

F32 = mybir.dt.float32
BF16 = mybir.dt.bfloat16

N_CORES = 8
B_FULL = 1024
BPC = B_FULL // N_CORES      # 128 batch per core
C = 256
H = 8
DPH = 32
DIM = 16
L = 8
S = L * L                     # 64
SCALE = DPH ** -0.5
B_BLK = 32                    # batch block
N_BLK = BPC // B_BLK          # 4 blocks
N_SUB = B_BLK // 8            # 8-b subblocks per block

bf = ml_dtypes.bfloat16


def _build_consts(Wq, Wk, Wv, Wo, emb_h, emb_w):
    """Host-side constant preparation (all bf16)."""
    Wq = np.asarray(Wq, np.float32) * SCALE
    Wk = np.asarray(Wk, np.float32)
    Wv = np.asarray(Wv, np.float32)
    Wo = np.asarray(Wo, np.float32)
    emb_h = np.asarray(emb_h, np.float32)
    emb_w = np.asarray(emb_w, np.float32)

    def chunked(W):
        # [C, 256] -> SBUF tile layout [c_in_chunk 128, (kchunk 2, dout 256)]
        return np.ascontiguousarray(
            W.reshape(2, 128, 256).transpose(1, 0, 2).reshape(128, 512)).astype(bf)

    consts = {
        "wq": chunked(Wq), "wk": chunked(Wk), "wv": chunked(Wv),
        "wo": chunked(Wo),
    }
    # skew weights: WH[i] [C, 64=(n,u)] = sum_{d<16} Wq_s[c, n*32+d]*emb_h[u+7-i, d]
    WH = np.zeros((L, C, 64), np.float32)
    WW = np.zeros((L, C, 64), np.float32)
    for i in range(L):
        for n in range(H):
            for u in range(8):
                WH[i, :, n * 8 + u] = Wq[:, n * 32:n * 32 + DIM] @ emb_h[u + 7 - i]
                WW[i, :, n * 8 + u] = Wq[:, n * 32 + DIM:n * 32 + 32] @ emb_w[u + 7 - i]
    consts["wh"] = np.ascontiguousarray(
        WH.reshape(L, 2, 128, 64).transpose(2, 0, 1, 3).reshape(128, L * 2 * 64)
    ).astype(bf)
    consts["ww"] = np.ascontiguousarray(
        WW.reshape(L, 2, 128, 64).transpose(2, 0, 1, 3).reshape(128, L * 2 * 64)
    ).astype(bf)
    # Sel_n [128, 64]: rows n*8+u -> δ(u==p); rows 64+n*8+v -> δ(v==q)
    Sel = np.zeros((H, 128, 64), np.float32)
    for n in range(H):
        for p in range(L):
            for q in range(L):
                Sel[n, n * 8 + p, p * 8 + q] = 1.0
                Sel[n, 64 + n * 8 + q, p * 8 + q] = 1.0
    consts["sel"] = np.ascontiguousarray(
        Sel.transpose(1, 0, 2).reshape(128, H * 64)).astype(bf)
    consts["ident"] = np.eye(128, dtype=np.float32).astype(bf)
    consts["ones"] = np.ones((64, 64), np.float32).astype(bf)
    return consts


def _build_program():
    nc = bacc.Bacc("TRN2", target_bir_lowering=False, debug=False,
                   num_devices=N_CORES)
    x_d = nc.dram_tensor("x", [BPC, C, S], F32, kind="ExternalInput").ap()
    wq_d = nc.dram_tensor("wq", [128, 512], BF16, kind="ExternalInput").ap()
    wk_d = nc.dram_tensor("wk", [128, 512], BF16, kind="ExternalInput").ap()
    wv_d = nc.dram_tensor("wv", [128, 512], BF16, kind="ExternalInput").ap()
    wo_d = nc.dram_tensor("wo", [128, 512], BF16, kind="ExternalInput").ap()
    wh_d = nc.dram_tensor("wh", [128, L * 2 * 64], BF16, kind="ExternalInput").ap()
    ww_d = nc.dram_tensor("ww", [128, L * 2 * 64], BF16, kind="ExternalInput").ap()
    sel_d = nc.dram_tensor("sel", [128, H * 64], BF16, kind="ExternalInput").ap()
    id_d = nc.dram_tensor("ident", [128, 128], BF16, kind="ExternalInput").ap()
    on_d = nc.dram_tensor("ones", [64, 64], BF16, kind="ExternalInput").ap()
    out_d = nc.dram_tensor("out", [BPC, C, S], F32, kind="ExternalOutput").ap()

    with tile.TileContext(nc) as tc:
        _kernel_body(tc, nc, x_d, wq_d, wk_d, wv_d, wo_d, wh_d, ww_d, sel_d,
                     id_d, on_d, out_d)
    nc.compile()
    return nc


def _kernel_body(tc, nc, x_d, wq_d, wk_d, wv_d, wo_d, wh_d, ww_d, sel_d,
                 id_d, on_d, out_d):
    from contextlib import ExitStack
    ctx = ExitStack()
    const_p = ctx.enter_context(tc.tile_pool(name="const", bufs=1))
    x_p = ctx.enter_context(tc.tile_pool(name="xp", bufs=2))
    act_p = ctx.enter_context(tc.tile_pool(name="actp", bufs=2))
    e_p = ctx.enter_context(tc.tile_pool(name="ep", bufs=4))
    out_p = ctx.enter_context(tc.tile_pool(name="outp", bufs=2))
    mm_ps = ctx.enter_context(tc.tile_pool(name="mmps", bufs=3, space="PSUM"))
    lg_ps = ctx.enter_context(tc.tile_pool(name="lgps", bufs=2, space="PSUM"))
    sm_ps = ctx.enter_context(tc.tile_pool(name="smps", bufs=1, space="PSUM"))
    at_ps = ctx.enter_context(tc.tile_pool(name="atps", bufs=1, space="PSUM"))

    # ---- constants to SBUF ----
    wq_s = const_p.tile([128, 512], BF16, name="wq_s")
    wk_s = const_p.tile([128, 512], BF16, name="wk_s")
    wv_s = const_p.tile([128, 512], BF16, name="wv_s")
    wo_s = const_p.tile([128, 512], BF16, name="wo_s")
    for t, d in ((wq_s, wq_d), (wk_s, wk_d), (wv_s, wv_d), (wo_s, wo_d)):
        nc.sync.dma_start(t[:], d)
    wh_s = const_p.tile([128, L * 2 * 64], BF16, name="wh_s")
    ww_s = const_p.tile([128, L * 2 * 64], BF16, name="ww_s")
    nc.sync.dma_start(wh_s[:], wh_d)
    nc.sync.dma_start(ww_s[:], ww_d)
    sel_s = const_p.tile([128, H * 64], BF16, name="sel_s")
    nc.sync.dma_start(sel_s[:], sel_d)
    id_s = const_p.tile([128, 128], BF16, name="id_s")
    nc.sync.dma_start(id_s[:], id_d)
    ones_s = const_p.tile([128, 64], BF16, name="ones_s")
    nc.sync.dma_start(ones_s[0:64, :], on_d)
    nc.sync.dma_start(ones_s[64:128, :], on_d)

    xv = x_d.rearrange("b c s -> c b s")            # [256, BPC, 64]
    ov = out_d.rearrange("b c s -> c b s")

    def W_lhsT(wt, k, m):
        return wt[:, k * 256 + m * 128: k * 256 + (m + 1) * 128]

    for blk in range(N_BLK):
        b0 = blk * B_BLK
        FB = B_BLK * S                              # 2048 free per chunk
        with nc.named_scope(f"blk{blk}_load"):
            xf = [x_p.tile([128, FB], F32, name="xf", tag="xf") for _ in range(2)]
            for kc in range(2):
                nc.sync.dma_start(
                    xf[kc].rearrange("c (b s) -> c b s", s=S),
                    xv[kc * 128:(kc + 1) * 128, b0:b0 + B_BLK, :])
            xb = [x_p.tile([128, FB], BF16, name="xb", tag="xb") for _ in range(2)]
            for kc in range(2):
                eng = nc.vector if kc == 0 else nc.scalar
                if kc == 0:
                    nc.vector.tensor_copy(xb[kc][:], xf[kc][:])
                else:
                    nc.scalar.copy(xb[kc][:], xf[kc][:])

        # ---- projections q/k/v (per 8-b sub, N=512) ----
        qT = [act_p.tile([128, FB], BF16, name="qT", tag="qT") for _ in range(2)]
        kT = [act_p.tile([128, FB], BF16, name="kT", tag="kT") for _ in range(2)]
        vT = [act_p.tile([128, FB], BF16, name="vT", tag="vT") for _ in range(2)]
        with nc.named_scope(f"blk{blk}_proj"):
            for sub in range(N_SUB):
                fs = slice(sub * 512, (sub + 1) * 512)
                for wt, dst in ((wq_s, qT), (wk_s, kT), (wv_s, vT)):
                    for mc in range(2):
                        ps = mm_ps.tile([128, 512], F32, name="pps", tag="pps")
                        for kc in range(2):
                            nc.tensor.matmul(ps[:], W_lhsT(wt, kc, mc),
                                             xb[kc][:, fs],
                                             start=(kc == 0), stop=(kc == 1))
                        eng_i = (sub + mc) % 2
                        if eng_i == 0:
                            nc.vector.tensor_copy(dst[mc][:, fs], ps[:])
                        else:
                            nc.scalar.copy(dst[mc][:, fs], ps[:])

        # ---- skew projections (per i / per j, N=256) ----
        skew = act_p.tile([128, B_BLK * S], BF16, name="skew", tag="skew")
        with nc.named_scope(f"blk{blk}_skew"):
            for i in range(L):
                # H part: free = (b, j) for fixed i -> strided dest
                ps = mm_ps.tile([128, 512], F32, name="pps", tag="pps")
                for kc in range(2):
                    nc.tensor.matmul(
                        ps[0:64, 0:256],
                        wh_s[:, (i * 2 + kc) * 64:(i * 2 + kc) * 64 + 64],
                        xb[kc].rearrange("c (b s) -> c b s", s=S)[:, :, i * 8:i * 8 + 8],
                        start=(kc == 0), stop=(kc == 1))
                for kc in range(2):
                    nc.tensor.matmul(
                        ps[64:128, 0:256],
                        ww_s[:, (i * 2 + kc) * 64:(i * 2 + kc) * 64 + 64],
                        xb[kc].rearrange("c (b s) -> c b s", s=S)[:, :, i:S:8],
                        start=(kc == 0), stop=(kc == 1), tile_position=(0, 64))
                # scatter copy to skew sbuf: H rows 0-63 at s=i*8+j, W rows 64+ at s=j*8+i
                dstH = skew.rearrange("p (b s) -> p b s", s=S)[0:64, :, i * 8:i * 8 + 8]
                dstW = skew.rearrange("p (b s) -> p b s", s=S)[64:128, :, i:S:8]
                srcH = ps[:, 0:256].rearrange("p (b s) -> p b s", s=8)[0:64]
                srcW = ps[:, 0:256].rearrange("p (b s) -> p b s", s=8)[64:128]
                if i % 2 == 0:
                    nc.vector.tensor_copy(dstH, srcH)
                    nc.scalar.copy(dstW, srcW)
                else:
                    nc.scalar.copy(dstH, srcH)
                    nc.vector.tensor_copy(dstW, srcW)

        # ---- V transpose (PE); two zero-padded parity variants ----
        vPg = [act_p.tile([128, B_BLK // 2 * 256], BF16, name=f"vPg{g}",
                          tag=f"vPg{g}") for g in range(2)]
        with nc.named_scope(f"blk{blk}_vt"):
            nc.vector.memset(vPg[0][64:128, :], 0.0)
            nc.scalar.memzero(vPg[1][0:64, :])
            for bp in range(B_BLK // 2):
                for chk in range(2):
                    pst = at_ps.tile([128, 128], BF16, name="tps", tag="tps")
                    nc.tensor.transpose(pst[:], vT[chk][:, bp * 128:(bp + 1) * 128],
                                        id_s[:])
                    fs = slice(bp * 256 + chk * 128, bp * 256 + (chk + 1) * 128)
                    if (bp + chk) % 2 == 0:
                        nc.vector.tensor_copy(vPg[0][0:64, fs], pst[0:64, :])
                        nc.scalar.copy(vPg[1][64:128, fs], pst[64:128, :])
                    else:
                        nc.scalar.copy(vPg[0][0:64, fs], pst[0:64, :])
                        nc.vector.tensor_copy(vPg[1][64:128, fs], pst[64:128, :])

        # ---- logits / softmax / WV / O-proj per b-pair windows ----
        attnT = [out_p.tile([128, FB], BF16, name="attnT", tag="attnT")
                 for _ in range(2)]
        with nc.named_scope(f"blk{blk}_attn"):
            for bp in range(B_BLK // 2):
                lg = lg_ps.tile([128, 512], F32, name="lg", tag="lg")
                for n in range(H):
                    kc, r0 = n // 4, (n % 4) * 32
                    for g in range(2):
                        b_loc = bp * 2 + g
                        fb = slice(b_loc * S, (b_loc + 1) * S)
                        # CC: lhsT=k [32,64] cols=pq ; rhs=q [32,64] cols=ij
                        nc.tensor.matmul(
                            lg[g * 64:(g + 1) * 64, n * 64:(n + 1) * 64],
                            kT[kc][r0:r0 + 32, fb], qT[kc][r0:r0 + 32, fb],
                            start=True, stop=False, tile_position=(r0, g * 64),
                            skip_group_check=True)
                        # CP: lhsT=Sel_n [128,64] ; rhs=skew [128,64]
                        nc.tensor.matmul(
                            lg[g * 64:(g + 1) * 64, n * 64:(n + 1) * 64],
                            sel_s[:, n * 64:(n + 1) * 64], skew[:, fb],
                            start=False, stop=True, tile_position=(0, g * 64),
                            skip_group_check=True)
                # exp
                E = e_p.tile([128, 512], BF16, name="E", tag="E")
                nc.scalar.activation(E[:], lg[:], mybir.ActivationFunctionType.Exp)
                # sums (replicated) via ones-matmul, then reciprocal, then norm
                sm = sm_ps.tile([128, 512], F32, name="sm", tag="sm")
                for g in range(2):
                    nc.tensor.matmul(sm[g * 64:(g + 1) * 64, :],
                                     ones_s[g * 64:(g + 1) * 64, :],
                                     E[g * 64:(g + 1) * 64, :],
                                     start=True, stop=True,
                                     tile_position=(g * 64, g * 64))
                rE = e_p.tile([128, 512], F32, name="rE", tag="rE")
                nc.vector.reciprocal(rE[:], sm[:])
                En = e_p.tile([128, 512], BF16, name="En", tag="En")
                nc.vector.tensor_mul(En[:], E[:], rE[:])
                # WV: per (g, n): lhsT=vP [64,32] ; rhs=En [64,64] -> attnT psum
                for chk in range(2):
                    at = at_ps.tile([128, 128], F32, name="at", tag="at")
                    for g in range(2):
                        for nn in range(4):
                            n = chk * 4 + nn
                            lhs = vPg[g][:, bp * 256 + chk * 128 + nn * 32:
                                         bp * 256 + chk * 128 + (nn + 1) * 32]
                            nc.tensor.matmul(
                                at[nn * 32:(nn + 1) * 32, g * 64:(g + 1) * 64],
                                lhs, En[:, n * 64:(n + 1) * 64],
                                start=True, stop=True,
                                tile_position=(0, nn * 32),
                                skip_group_check=True)
                    dst = attnT[chk][:, bp * 128:(bp + 1) * 128]
                    if chk == 0:
                        nc.vector.tensor_copy(dst, at[:])
                    else:
                        nc.scalar.copy(dst, at[:])

        # ---- O-projection ----
        with nc.named_scope(f"blk{blk}_oproj"):
            for sub in range(N_SUB):
                fs = slice(sub * 512, (sub + 1) * 512)
                for mc in range(2):
                    ps = mm_ps.tile([128, 512], F32, name="pps", tag="pps")
                    for kc in range(2):
                        nc.tensor.matmul(ps[:], W_lhsT(wo_s, kc, mc),
                                         attnT[kc][:, fs],
                                         start=(kc == 0), stop=(kc == 1))
                    ot = out_p.tile([128, 512], F32, name="ot", tag="ot")
                    if (sub + mc) % 2 == 0:
                        nc.vector.tensor_copy(ot[:], ps[:])
                    else:
                        nc.scalar.copy(ot[:], ps[:])
                    nc.sync.dma_start(
                        ov[mc * 128:(mc + 1) * 128, b0 + sub * 8:b0 + (sub + 1) * 8, :],
                        ot.rearrange("c (b s) -> c b s", s=S))
    ctx.close()


_NC_CACHE = None


def kernel(x, Wq, Wk, Wv, Wo, emb_h, emb_w):
    global _NC_CACHE
    x = np.asarray(x, np.float32)
    B = x.shape[0]
    consts = _build_consts(Wq, Wk, Wv, Wo, emb_h, emb_w)
    if _NC_CACHE is None:
        _NC_CACHE = _build_program()
    nc = _NC_CACHE
    in_maps = []
    for c in range(N_CORES):
        m = {"x": np.ascontiguousarray(
            x[c * BPC:(c + 1) * BPC].reshape(BPC, C, S))}
        m.update(consts)
        in_maps.append(m)
    res = run_bass_kernel_spmd(nc, in_maps, core_ids=list(range(N_CORES)))
    out = np.concatenate([res.results[c]["out"] for c in range(N_CORES)], axis=0)
    return out.reshape(B, C, L, L).astype(np.float32)
